# revision 11
# baseline (speedup 1.0000x reference)
"""Multi-level (FPN) DeformRoIPool (zero-offset == aligned RoIAlign) for Trainium2.

Strategy (8 NeuronCores, SPMD, one Bass program):
- Host computes, per ROI, the set of DISTINCT feature pixels its 7x7x2x2
  bilinear sample grid touches (a Y x X grid product, ~0.37x the naive
  per-sample corner count) and lays them out channels-last as dense rows.
- ROIs are sorted by pixel count and dealt round-robin to the 8 cores so
  every core gets an identical slot-size profile -> one SPMD program.
- The bilinear+average reduction weight matrix factors as kron(Ay, Ax)
  ([nPix, 49] per ROI); host bakes it into per-128-row-group weight tiles.
- Device: big sequential HWDGE DMAs (no gather, no GpSimd), one matmul per
  (row-group, roi) set accumulating [49 bins, 256 ch] in PSUM, DVE cast to
  fp16, DMA out. Memory-bound by ~5-7 MB/core of HBM reads.
"""
import numpy as np

OUT = 7
SR = 2
STRIDES = (4, 8, 16, 32)
FINEST = 56.0
NLEV = 4
C = 256
NBIN = OUT * OUT
N_ROIS = 256
N_CORES = 8
NROI_C = N_ROIS // N_CORES          # 32 roi slots per core
FEAT_SHAPES = [(2, 256, 200, 200), (2, 256, 100, 100), (2, 256, 50, 50), (2, 256, 25, 25)]


# ---------------------------------------------------------------------------
# BIR fix: this container's walrus rejects >1 embedded sem wait per
# instruction (2 on EventSemaphore). Split excess waits onto EventSemaphore
# carriers at serialization time.
# ---------------------------------------------------------------------------
def _install_bir_waitsplit():
    import orjson
    import concourse.bass as bass

    if getattr(bass.Bass, "_waitsplit_patched", False):
        return

    def _fix_blocks(blocks, counter):
        for blk in blocks:
            insts = blk.get("instructions")
            if insts:
                out = []
                for ins in insts:
                    si = ins.get("sync_info")
                    ow = (si or {}).get("on_wait") or []
                    limit = 2 if ins.get("opcode") == "EventSemaphore" else 1
                    if len(ow) > limit:
                        excess = ow[: len(ow) - limit]
                        si["on_wait"] = ow[len(ow) - limit:]
                        for i in range(0, len(excess), 2):
                            counter[0] += 1
                            out.append({
                                "name": f"I-waitsplit-{counter[0]}",
                                "opcode": "EventSemaphore",
                                "engine": ins["engine"],
                                "ins": [], "outs": [],
                                "debug": ins.get("debug", 0),
                                "sync_info": {"on_update": [], "on_wait": excess[i:i + 2]},
                            })
                    out.append(ins)
                blk["instructions"] = out
            if blk.get("blocks"):
                _fix_blocks(blk["blocks"], counter)

    orig = bass.Bass.to_json_bytes

    def to_json_bytes(self, *a, **kw):
        data = orig(self, *a, **kw)
        d = orjson.loads(data)
        counter = [0]
        for fn in d.get("functions", []):
            _fix_blocks(fn.get("blocks", []), counter)
        return orjson.dumps(d) if counter[0] else data

    bass.Bass.to_json_bytes = to_json_bytes
    bass.Bass._waitsplit_patched = True


# ---------------------------------------------------------------------------
# Host-side layout computation
# ---------------------------------------------------------------------------
def _roi_meta(rois, feat_shapes):
    """Per-roi level + distinct pixel grid + separable weight factors.

    The reduction out[b=(i,jj), c] = sum_s w_s * F(sample corners)_c over the
    7x7x2x2 grid factors per ROI as kron(Ay, Ax): Ay[line, i], Ax[col, jj].
    """
    scale_wh = np.sqrt((rois[:, 3] - rois[:, 1]) * (rois[:, 4] - rois[:, 2]))
    with np.errstate(divide="ignore"):
        tl = np.clip(np.floor(np.log2(scale_wh / FINEST + 1e-6)), 0, NLEV - 1)
    tl = (tl + 1e-5).astype(np.int32)
    g = np.arange(OUT, dtype=np.float64)[:, None] + (np.arange(SR, dtype=np.float64)[None, :] + 0.5) / SR

    def axis_factor(lo, ext, L):
        """1D positions lo + ext/OUT * g -> (lines, A[nl, OUT]) weight factor."""
        p = lo + (ext / OUT) * g                    # [OUT, SR]
        v = (p > -1) & (p < L)
        pc = np.clip(p, 0.0, L - 1)
        p0 = np.minimum(np.floor(pc).astype(np.int64), L - 1)
        p1 = np.minimum(p0 + 1, L - 1)
        fr = pc - p0
        lines = np.unique(np.concatenate([p0.ravel(), p1.ravel()]))
        r0 = np.searchsorted(lines, p0)
        r1 = np.searchsorted(lines, p1)
        A = np.zeros((len(lines), OUT), np.float64)
        w0 = (1.0 - fr) * v / SR
        w1 = fr * v / SR
        for i in range(OUT):
            for s in range(SR):
                A[r0[i, s], i] += w0[i, s]
                A[r1[i, s], i] += w1[i, s]
        return lines, A

    metas = []
    for n in range(rois.shape[0]):
        l = int(tl[n])
        _, _, H, W = feat_shapes[l]
        sc = 1.0 / STRIDES[l]
        x1 = rois[n, 1] * sc - 0.5
        y1 = rois[n, 2] * sc - 0.5
        rw = rois[n, 3] * sc - 0.5 - x1
        rh = rois[n, 4] * sc - 0.5 - y1
        ylines, Ay = axis_factor(y1, rh, H)
        xlines, Ax = axis_factor(x1, rw, W)
        metas.append(dict(
            l=l, b=int(rois[n, 0]),
            ylines=ylines, xlines=xlines, Ay=Ay, Ax=Ax,
            npix=len(ylines) * len(xlines),
        ))
    return metas


def _build_layout(metas):
    """Uniform-across-cores slot layout.

    Sort rois by npix, deal round-robin: core c, slot k -> roi order[8k+c].
    Slot budget R_k = max npix over the octet -> identical structure per core.
    """
    order = np.argsort([-m["npix"] for m in metas], kind="stable")
    budgets = []
    for k in range(NROI_C):
        octet = [metas[order[k * N_CORES + c]]["npix"] for c in range(N_CORES)]
        budgets.append(max(octet))
    offs = np.concatenate([[0], np.cumsum(budgets)]).astype(np.int64)
    r_total = int(offs[-1])
    ngrp = -(-r_total // 128)
    r_pad = ngrp * 128

    # sets: for each 128-row group, one matmul per slot whose budget range
    # intersects it. start/stop flag per slot's first/last set.
    sets = []           # (group, slot)
    first, last = {}, {}
    for gidx in range(ngrp):
        lo, hi = gidx * 128, gidx * 128 + 128
        for k in range(NROI_C):
            if offs[k] < hi and offs[k + 1] > lo:
                si = len(sets)
                sets.append((gidx, k))
                first.setdefault(k, si)
                last[k] = si
    return dict(order=order, budgets=budgets, offs=offs, r_total=r_total,
                ngrp=ngrp, r_pad=r_pad, sets=sets, first=first, last=last)


def _build_core_data(feats_T, metas, layout, core):
    """G rows + weight tiles for one core, in partition-major device layout."""
    ngrp, offs = layout["ngrp"], layout["offs"]
    sets = layout["sets"]
    G = np.zeros((layout["r_pad"], C), np.float32)
    W = np.zeros((len(sets), 128, NBIN), np.float32)
    for k in range(NROI_C):
        m = metas[layout["order"][k * N_CORES + core]]
        fT = feats_T[m["l"]][m["b"]]                     # [H, W, C]
        ny, nx = len(m["ylines"]), len(m["xlines"])
        G[offs[k]:offs[k] + ny * nx] = fT[m["ylines"]][:, m["xlines"]].reshape(-1, C)
    for si, (gidx, k) in enumerate(sets):
        m = metas[layout["order"][k * N_CORES + core]]
        ny, nx = len(m["ylines"]), len(m["xlines"])
        npix = ny * nx
        lo = max(gidx * 128, int(offs[k]))
        hi = min(gidx * 128 + 128, int(offs[k]) + npix)
        if hi <= lo:
            continue
        pix = np.arange(lo - offs[k], hi - offs[k])
        wy = m["Ay"][pix // nx]                          # [np, 7]
        wx = m["Ax"][pix % nx]                           # [np, 7]
        W[si, lo - gidx * 128:hi - gidx * 128] = np.einsum(
            "pi,pj->pij", wy, wx).reshape(-1, NBIN)
    # partition-major: row r -> [r % 128, r // 128]
    Gp = np.ascontiguousarray(
        G.reshape(ngrp, 128, C).transpose(1, 0, 2).reshape(128, ngrp * C)
    ).astype(np.float16)
    Wp = np.ascontiguousarray(
        W.transpose(1, 0, 2).reshape(128, len(sets) * NBIN)
    ).astype(np.float16)
    return Gp, Wp


# ---------------------------------------------------------------------------
# Device program
# ---------------------------------------------------------------------------
def _build_program(layout):
    import concourse.bacc as bacc
    import concourse.mybir as mybir
    import concourse.tile as tile

    _install_bir_waitsplit()
    nc = bacc.Bacc("TRN2", debug=False, enable_asserts=True, num_devices=N_CORES)

    ngrp = layout["ngrp"]
    sets = layout["sets"]
    nsets = len(sets)
    first, last = layout["first"], layout["last"]

    g_d = nc.dram_tensor("g", [128, ngrp * C], mybir.dt.float16, kind="ExternalInput")
    w_d = nc.dram_tensor("w", [128, nsets * NBIN], mybir.dt.float16, kind="ExternalInput")
    out_d = nc.dram_tensor("out", [NBIN, NROI_C * C], mybir.dt.float16, kind="ExternalOutput")

    # graded chunks on group boundaries: small first chunks start the PE
    # early, all tiles resident (no reuse stalls), FIFO drain on one ring
    sizes = []
    rem, sz = ngrp, 3
    while rem > 0:
        take = min(sz, rem)
        sizes.append(take)
        rem -= take
        sz = min(8, sz + 2)
    bounds = np.concatenate([[0], np.cumsum(sizes)]).astype(int)
    chunks = [(int(bounds[i]), int(bounds[i + 1])) for i in range(len(sizes))]
    # weights in slices on the scalar HWDGE ring (parallel to G on sync)
    w_splits = [0]
    for ci in (0, 1, 3):
        si = w_splits[-1]
        while si < nsets and sets[si][0] < chunks[min(ci, len(chunks) - 1)][1]:
            si += 1
        w_splits.append(si)
    w_splits.append(nsets)
    w_splits = sorted(set(w_splits))

    with tile.TileContext(nc) as tc:
        with (
            tc.tile_pool(name="wp", bufs=1) as wp,
            tc.tile_pool(name="gp", bufs=1) as gp,
            tc.tile_pool(name="sp", bufs=1) as sp,
            tc.tile_pool(name="pp", bufs=8, space="PSUM") as pp,
        ):
            st = sp.tile([NBIN, NROI_C * C], mybir.dt.float16)
            wt = wp.tile([128, nsets * NBIN], mybir.dt.float16)
            gts = []
            for ci, (c0, c1) in enumerate(chunks):
                gt = gp.tile([128, (c1 - c0) * C], mybir.dt.float16, name=f"g{ci}", tag=f"g{ci}")
                nc.sync.dma_start(gt[:], g_d[:, c0 * C:c1 * C])
                gts.append(gt)
                if ci == 0:
                    for s0, s1 in zip(w_splits[:-1], w_splits[1:]):
                        nc.scalar.dma_start(
                            wt[:, s0 * NBIN:s1 * NBIN], w_d[:, s0 * NBIN:s1 * NBIN]
                        )
            ps_of = {}
            si = 0
            for ci, (c0, c1) in enumerate(chunks):
                gt = gts[ci]
                while si < len(sets) and sets[si][0] < c1:
                    gidx, k = sets[si]
                    if si == first[k]:
                        ps_of[k] = pp.tile([NBIN, C], mybir.dt.float32, tag="ps", name=f"ps_{k}")
                    nc.tensor.matmul(
                        out=ps_of[k][:, :],
                        lhsT=wt[:, si * NBIN:(si + 1) * NBIN],
                        rhs=gt[:, (gidx - c0) * C:(gidx - c0 + 1) * C],
                        start=(si == first[k]),
                        stop=(si == last[k]),
                    )
                    if si == last[k]:
                        # alternate engines for the late (tiny-roi) casts so the
                        # serial cast tail halves; ACT can read PSUM too
                        if k >= 24 and k % 2:
                            nc.scalar.copy(st[:, k * C:(k + 1) * C], ps_of[k][:])
                        else:
                            nc.vector.tensor_copy(st[:, k * C:(k + 1) * C], ps_of[k][:])
                    si += 1
            for o0 in range(0, NROI_C, 8):
                nc.scalar.dma_start(
                    out_d[:, o0 * C:(o0 + 8) * C], st[:, o0 * C:(o0 + 8) * C]
                )
    nc.compile()
    return nc


# ---------------------------------------------------------------------------
# Entry point
# ---------------------------------------------------------------------------
def kernel(feat0, feat1, feat2, feat3, rois):
    from concourse.bass_utils import run_bass_kernel_spmd

    feats = [np.asarray(f, np.float32) for f in (feat0, feat1, feat2, feat3)]
    rois = np.asarray(rois, np.float32)
    feat_shapes = [f.shape for f in feats]
    feats_T = [np.ascontiguousarray(f.transpose(0, 2, 3, 1)) for f in feats]
    metas = _roi_meta(rois, feat_shapes)
    layout = _build_layout(metas)

    in_maps = []
    for core in range(N_CORES):
        Gp, Wp = _build_core_data(feats_T, metas, layout, core)
        in_maps.append({"g": Gp, "w": Wp})

    nc = _build_program(layout)
    res = run_bass_kernel_spmd(nc, in_maps, core_ids=list(range(N_CORES)), trace=False)

    out = np.zeros((N_ROIS, C, OUT, OUT), np.float32)
    order = layout["order"]
    for core in range(N_CORES):
        o = res.results[core]["out"].astype(np.float32).reshape(NBIN, NROI_C, C)
        for k in range(NROI_C):
            out[order[k * N_CORES + core]] = o[:, k].T.reshape(C, OUT, OUT)
    return out


# Testing hook: emulate the device math in numpy (same G/W data).
def emulate(feat0, feat1, feat2, feat3, rois):
    feats = [np.asarray(f, np.float32) for f in (feat0, feat1, feat2, feat3)]
    rois = np.asarray(rois, np.float32)
    feat_shapes = [f.shape for f in feats]
    feats_T = [np.ascontiguousarray(f.transpose(0, 2, 3, 1)) for f in feats]
    metas = _roi_meta(rois, feat_shapes)
    layout = _build_layout(metas)
    sets = layout["sets"]
    out = np.zeros((N_ROIS, C, OUT, OUT), np.float32)
    for core in range(N_CORES):
        Gp, Wp = _build_core_data(feats_T, metas, layout, core)
        Gf = Gp.astype(np.float32).reshape(128, layout["ngrp"], C)
        Wf = Wp.astype(np.float32).reshape(128, len(sets), NBIN)
        acc = {k: np.zeros((NBIN, C), np.float32) for k in range(NROI_C)}
        for si, (gidx, k) in enumerate(sets):
            acc[k] += Wf[:, si, :].T @ Gf[:, gidx, :]
        for k in range(NROI_C):
            r = layout["order"][k * N_CORES + core]
            out[r] = acc[k].astype(np.float16).astype(np.float32).T.reshape(C, OUT, OUT)
    return out
